# revision 1
# baseline (speedup 1.0000x reference)
"""CrossCovarianceAttn kernel — data-parallel over batch across 8 NeuronCores.

Shapes (hardcoded per spec): x [16, 3136, 768] f32, qkv_w [768, 2304],
temperature [16, 1, 1], proj_w [768, 768], proj_b [768].
Each core processes B/8 = 2 batches; weights are replicated. No collectives
needed: every stage (qkv proj, per-head [D,D] cross-covariance attention,
output proj) is batch-independent.
"""
import sys

sys.path.insert(0, "/opt/trn_rl_repo")
sys.path.insert(0, "/root/.axon_site/_ro/trn_rl_repo")

import numpy as np

B, N, C = 16, 3136, 768
H = 16
D = C // H
EPS = 1e-12
NCORES = 8

_compiled = {}


def _attn_block(x, qkv_w, temperature, proj_w, proj_b):
    import jax.numpy as jnp

    Bl = x.shape[0]
    qkv = (x @ qkv_w).reshape(Bl, N, 3, H, D).transpose(2, 0, 3, 4, 1)
    q, k, v = qkv[0], qkv[1], qkv[2]  # [Bl, H, D, N]

    def l2n(t):
        n = jnp.sqrt(jnp.sum(t * t, axis=-1, keepdims=True))
        return t / jnp.maximum(n, EPS)

    qn = l2n(q)
    kn = l2n(k)
    attn = jnp.einsum("bhdn,bhen->bhde", qn, kn) * temperature
    attn = jax.nn.softmax(attn, axis=-1)
    out = jnp.einsum("bhde,bhen->bhdn", attn, v)
    out = out.transpose(0, 3, 1, 2).reshape(Bl, N, C)
    return out @ proj_w + proj_b


try:
    import jax
except Exception:  # pragma: no cover
    jax = None


def _get_pmapped():
    if "fn" not in _compiled:
        _compiled["fn"] = jax.pmap(
            _attn_block,
            in_axes=(0, None, None, None, None),
            devices=jax.devices()[:NCORES],
        )
    return _compiled["fn"]


def kernel(x, qkv_w, temperature, proj_w, proj_b):
    x = np.asarray(x, dtype=np.float32)
    qkv_w = np.asarray(qkv_w, dtype=np.float32)
    temperature = np.asarray(temperature, dtype=np.float32)
    proj_w = np.asarray(proj_w, dtype=np.float32)
    proj_b = np.asarray(proj_b, dtype=np.float32)

    if jax is not None and len(jax.devices()) >= NCORES:
        try:
            xs = x.reshape(NCORES, B // NCORES, N, C)
            fn = _get_pmapped()
            out = fn(xs, qkv_w, temperature, proj_w, proj_b)
            return np.asarray(out).reshape(B, N, C).astype(np.float32)
        except Exception:
            pass

    # Fallback: single-device / numpy path (correctness safety net).
    out = np.empty((B, N, C), dtype=np.float32)
    for b in range(B):
        qkv = (x[b] @ qkv_w).reshape(N, 3, H, D).transpose(1, 2, 3, 0)
        q, k, v = qkv[0], qkv[1], qkv[2]  # [H, D, N]
        qn = q / np.maximum(np.sqrt((q * q).sum(-1, keepdims=True)), EPS)
        kn = k / np.maximum(np.sqrt((k * k).sum(-1, keepdims=True)), EPS)
        a = np.einsum("hdn,hen->hde", qn, kn) * temperature
        a = a - a.max(-1, keepdims=True)
        e = np.exp(a)
        a = e / e.sum(-1, keepdims=True)
        o = np.einsum("hde,hen->hdn", a, v)
        out[b] = o.transpose(2, 0, 1).reshape(N, C) @ proj_w + proj_b
    return out
